# revision 34
# baseline (speedup 1.0000x reference)
"""FFTEmbedding kernel for Trainium2 (8 NeuronCores, SPMD data-parallel over B).

Math: the reference computes, per (b, t):
    window = x_pad[b, t : t+W]                (causal window, W=256)
    spec   = rfft(window); feats = [spec.real, spec.imag]   (258)
    out    = feats @ weight.T + bias          (512)

The pipeline is linear in x, so it collapses to a causal 1-D convolution
with a precomputed (W=256, EMB=512) matrix:
    M2[w, e] = sum_k weight[e, k]*cos(2*pi*k*w/W) - weight[e, 129+k]*sin(2*pi*k*w/W)
    out[b, t, e] = sum_w x_pad[b, t+w] * M2[w, e] + bias[e]

Device mapping (per core: 2 batch rows, weights replicated):
  * "mega-Hankel" SBUF image Hank[p, c] = x_pad[b, p+c], materialized by DMAs
    whose source access pattern overlaps: partition p reads the contiguous run
    x_pad[b, p : p+...]. Hank[k, 128j+m] = x_pad[128j+k+m] is symmetric in
    (k, m), so a 128-column slice IS the pre-transposed matmul lhsT.
  * per 128-t output tile i: psum[128, 512] = Hank[:, 128i:+128].T @ W0
    + Hank[:, 128(i+1):+128].T @ W1   (fp16 operands, fp32 PSUM accum).
    Two tiles share one 2-bank PSUM pair, evacuated by a single DVE
    tensor_tensor add that fuses the bias and casts to fp16.
  * K=1 rank-1 bias matmuls are deliberately NOT used: tiny-K matmuls kept
    the PE clock throttled at 1.2 GHz (measured); with pure 128x128x512
    matmuls the stream runs at the warm 2.4 GHz rate.
  * output staged fp16 in SBUF, stored fp16 (halves the dominant HBM write
    stream); the host upcasts to fp32. Measured end-to-end rel err ~4e-4.
"""

import os
import sys

import numpy as np

_TRN_REPO = "/opt/trn_rl_repo"
if _TRN_REPO not in sys.path:
    sys.path.insert(0, _TRN_REPO)

B, T, W_SIZE, EMB = 16, 8192, 256, 512
N_CORES = 8
B_PER = B // N_CORES          # 2 batch rows per core
PAD = W_SIZE - 1              # 255 leading zeros
XP_LEN = T + PAD + 1          # 8448 (one trailing pad elem)
HANK_COLS = T + W_SIZE - 128  # 8320 mega-Hankel free dim
HANK_A = 1280                 # small head chunk so PE starts early
HANK_B = HANK_COLS - HANK_A
N_TILES = T // 128            # 64 output tiles of 128 t's per batch row
N_PAIRS = N_TILES // 2        # 32 PSUM pairs per batch row
PAIRS_PER_SUP = 2             # supertile = 2 pairs = 4 tiles = 512 t rows
N_SUP = N_PAIRS // PAIRS_PER_SUP

# module-level knobs (test.py pokes these)
TRACE = os.environ.get("KERNEL_TRACE", "0") == "1"
USE_DT = os.environ.get("KERNEL_DT", "fp16")      # matmul operand dtype
OUT_DT = os.environ.get("KERNEL_OUT_DT", "fp16")  # device output dtype
LAST_RESULT = None

_CACHE = {}


def _build_m2(weight: np.ndarray) -> np.ndarray:
    """(EMB, 258) projection -> (W, EMB) causal-conv matrix, in float64."""
    k = np.arange(W_SIZE // 2 + 1, dtype=np.float64)   # 129
    w = np.arange(W_SIZE, dtype=np.float64)            # 256
    ang = 2.0 * np.pi * np.outer(k, w) / W_SIZE        # (129, 256)
    f = np.concatenate([np.cos(ang), -np.sin(ang)], axis=0)  # (258, 256)
    m2 = (weight.astype(np.float64) @ f).T             # (256, EMB)
    return np.ascontiguousarray(m2, dtype=np.float64)


def _round_fp22(a: np.ndarray) -> np.ndarray:
    """Round fp32 -> fp22 (e8m13, the TensorE f32r operand precision)."""
    u = np.ascontiguousarray(a, dtype=np.float32).view(np.uint32)
    u = (u + np.uint32(0x200)) & np.uint32(0xFFFFFC00)
    return u.view(np.float32)


def _build_program():
    from concourse import bacc, mybir, tile
    from concourse.ap import AP

    f32 = mybir.dt.float32
    fin = {
        "fp16": mybir.dt.float16,
        "bf16": mybir.dt.bfloat16,
        "f32r": mybir.dt.float32r,
        "f32": f32,
    }[USE_DT]
    fout = {"fp16": mybir.dt.float16, "bf16": mybir.dt.bfloat16, "f32": f32}[OUT_DT]
    add = mybir.AluOpType.add

    nc = bacc.Bacc(target_bir_lowering=False)
    xpad_h = nc.declare_dram_parameter("xpad", [B_PER, XP_LEN], fin, isOutput=False)
    # w2 pre-packed on host to the SBUF layout: w2[p, h*EMB+e] = M2[128h+p, e]
    w2_h = nc.declare_dram_parameter("w2", [128, 2 * EMB], fin, isOutput=False)
    # bias duplicated to cover a 2-bank (1024-wide) PSUM pair
    biasf_h = nc.declare_dram_parameter("biasf", [1, 2 * EMB], f32, isOutput=False)
    bias16_h = nc.declare_dram_parameter("bias16", [1, 2 * EMB], fout, isOutput=False)
    out_h = nc.declare_dram_parameter("out", [B_PER, T, EMB], fout, isOutput=True)

    with tile.TileContext(nc) as tc:
        with (
            tc.tile_pool(name="hank", bufs=2) as hank_pool,
            tc.tile_pool(name="wpool", bufs=1) as w_pool,
            tc.tile_pool(name="cpool", bufs=1) as c_pool,
            tc.tile_pool(name="sup", bufs=6) as sup_pool,
            tc.tile_pool(name="psum", bufs=4, space="PSUM") as psum_pool,
        ):
            # DMA placement: SP HWDGE ring starts first and carries, in
            # arrival-need order, exactly what the PE needs first:
            # w0 -> hankA0 -> w1 -> hankB0a -> hankB0b, then all out-DMAs.
            # The ACT ring carries bias rows + batch-1 hanks. (SWDGE was
            # measured to deliver ~8us late; avoid it.)
            w01 = w_pool.tile([128, 2 * EMB], fin, tag="w01")
            w0 = w01[:, 0:EMB]
            w1 = w01[:, EMB : 2 * EMB]

            bias_row = c_pool.tile([1, 2 * EMB], f32, tag="bias_row")
            nc.scalar.dma_start(bias_row[:, :], biasf_h[:, :])
            b16_row = c_pool.tile([1, 2 * EMB], fout, tag="b16_row")
            nc.scalar.dma_start(b16_row[:, :], bias16_h[:, :])
            bias_bc = c_pool.tile([128, 2 * EMB], f32, tag="bias_bc")
            nc.gpsimd.partition_broadcast(bias_bc[:, :], bias_row[:, :])
            bias_bc16 = c_pool.tile([128, 2 * EMB], fout, tag="bias_bc16")
            nc.gpsimd.partition_broadcast(bias_bc16[:, :], b16_row[:, :])

            # PE pre-warm: the HAM clock gate needs ~3.4us of sustained PE
            # activity to lift the 1.2 GHz cold throttle. While the input
            # DMAs are in flight (~7us), run dummy matmuls on a zeroed
            # scratch tile into a scratch PSUM pair nobody reads; the real
            # matmul stream then starts at the warm 2.4 GHz rate.
            junk = c_pool.tile([128, EMB], fin, tag="junk")
            nc.vector.memset(junk[:, :], 0.0)
            ps_warm = psum_pool.tile([128, 2 * EMB], f32, tag="ps")
            for _ in range(15):
                nc.tensor.matmul(
                    ps_warm[:, 0:EMB], junk[:, 0:128], junk[:, :],
                    start=True, stop=True,
                )

            # chunk ladder sized so each chunk's DMA completion (incl the
            # ~2us receipt) lands before the PE consumes the previous chunks
            CHUNKS = [HANK_A, 512, 1024, HANK_COLS - HANK_A - 512 - 1024]
            OFFS = [sum(CHUNKS[:j]) for j in range(len(CHUNKS) + 1)]

            def alloc_hank(b):
                return [
                    hank_pool.tile([128, c], fin, tag=f"hk{j}", name=f"hk{j}_{b}")
                    for j, c in enumerate(CHUNKS)
                ]

            def load_hank(b, tiles, eng):
                for j, t in enumerate(tiles):
                    eng.dma_start(
                        t[:, :],
                        AP(xpad_h, b * XP_LEN + OFFS[j], [[1, 128], [1, CHUNKS[j]]]),
                    )

            # batch-0 chunks interleaved with the weight halves on the SP ring
            hanks = [alloc_hank(0), alloc_hank(1)]
            h0 = hanks[0]
            nc.sync.dma_start(w0, w2_h[:, 0:EMB])
            nc.sync.dma_start(h0[0][:, :], AP(xpad_h, 0, [[1, 128], [1, CHUNKS[0]]]))
            nc.sync.dma_start(w1, w2_h[:, EMB : 2 * EMB])
            for j in range(1, len(CHUNKS)):
                nc.sync.dma_start(
                    h0[j][:, :], AP(xpad_h, OFFS[j], [[1, 128], [1, CHUNKS[j]]])
                )
            # batch-1 hank loads are deferred into the batch-0 loop (emitted
            # after supertile 4) so they don't steal SDMA bandwidth from the
            # critical batch-0 chunks during kernel start

            def hank_slice(b, c):
                """lhsT for column-block c (128 cols starting at 128*c)."""
                lo = 128 * c
                for j in range(len(CHUNKS)):
                    if lo + 128 <= OFFS[j + 1]:
                        off = lo - OFFS[j]
                        return hanks[b][j][:, off : off + 128]
                raise AssertionError(c)

            qglob = 0
            for b in range(B_PER):
                for g in range(N_SUP):
                    if b == 0 and g == 5:
                        load_hank(1, hanks[1], nc.scalar)
                    sup = sup_pool.tile([128, 2 * PAIRS_PER_SUP * EMB], fout)
                    for pq in range(PAIRS_PER_SUP):
                        ps = psum_pool.tile([128, 2 * EMB], f32)  # 2 banks
                        for h in range(2):
                            i = (g * PAIRS_PER_SUP + pq) * 2 + h
                            pslice = ps[:, h * EMB : (h + 1) * EMB]
                            nc.tensor.matmul(
                                pslice, hank_slice(b, i), w0,
                                start=True, stop=False,
                            )
                            nc.tensor.matmul(
                                pslice, hank_slice(b, i + 1), w1,
                                start=False, stop=True,
                            )
                        dst = sup[:, pq * 2 * EMB : (pq + 1) * 2 * EMB]
                        if qglob % 5 in (1, 2, 4) and qglob < 62:
                            # ACT evacuates; DVE applies bias in cheap 16-bit
                            # 2x mode. Offloads ~half of the evacuation work.
                            nc.scalar.copy(dst, ps[:, :])
                            nc.vector.tensor_tensor(dst, dst, bias_bc16[:, :], add)
                        else:
                            # single DVE op: evacuate + bias + cast
                            nc.vector.tensor_tensor(dst, ps[:, :], bias_bc[:, :], add)
                        qglob += 1
                    # store supertile: out[b, 512g + 128v + p, e] <- sup[p, 512v+e]
                    dst_ap = out_h[
                        b, g * 512 : (g + 1) * 512, :
                    ].rearrange("(v p) e -> p v e", p=128)
                    nc.sync.dma_start(dst_ap, sup[:, :])

    nc.finalize()
    return nc


def _get_program():
    key = ("prog", USE_DT, OUT_DT)
    if key not in _CACHE:
        _CACHE[key] = _build_program()
    return _CACHE[key]


def kernel(x: np.ndarray, weight: np.ndarray, bias: np.ndarray) -> np.ndarray:
    global LAST_RESULT
    from concourse.bass_utils import run_bass_kernel_spmd

    x = np.asarray(x, dtype=np.float32)
    weight = np.asarray(weight, dtype=np.float32)
    bias = np.asarray(bias, dtype=np.float32)

    m2 = _build_m2(weight).astype(np.float32)
    xpad = np.zeros((B, XP_LEN), dtype=np.float32)
    xpad[:, PAD : PAD + T] = x
    # pack to the SBUF tile layout: w2[p, h*EMB+e] = M2[128h+p, e]
    w2_in = np.ascontiguousarray(
        m2.reshape(2, 128, EMB).transpose(1, 0, 2).reshape(128, 2 * EMB)
    )
    bias2 = np.ascontiguousarray(
        np.concatenate([bias, bias]).reshape(1, 2 * EMB).astype(np.float32)
    )

    import ml_dtypes

    np_in = {
        "fp16": np.float16,
        "bf16": ml_dtypes.bfloat16,
        "f32r": np.float32,
        "f32": np.float32,
    }[USE_DT]
    np_out = {"fp16": np.float16, "bf16": ml_dtypes.bfloat16, "f32": np.float32}[OUT_DT]
    if USE_DT == "f32r":
        w2_in = _round_fp22(w2_in)
        xpad = _round_fp22(xpad)
    else:
        w2_in = w2_in.astype(np_in)
        xpad = xpad.astype(np_in)

    nc = _get_program()
    in_maps = [
        {
            "xpad": np.ascontiguousarray(xpad[c * B_PER : (c + 1) * B_PER]),
            "w2": w2_in,
            "biasf": bias2,
            "bias16": np.ascontiguousarray(bias2.astype(np_out)),
        }
        for c in range(N_CORES)
    ]
    res = run_bass_kernel_spmd(nc, in_maps, list(range(N_CORES)), trace=TRACE)
    LAST_RESULT = res
    out = np.concatenate([res.results[c]["out"] for c in range(N_CORES)], axis=0)
    return np.ascontiguousarray(out.astype(np.float32))
